# revision 37
# baseline (speedup 1.0000x reference)
"""ArcFace combined-margin loss kernel for 8 TRN2 NeuronCores.

Strategy
--------
reference: cos = (f @ w.T) / (|f||w|); phi = arcface(cos);
outputs = s*(labels*phi + (1-labels)*cos); loss = mean over rows of
-(sum of log_softmax(outputs) at lab_pinds, masked) / L^2.

labels is the multi-hot of (lab_pinds, lengths), so outputs differs from
s*cos only at <=8 entries/row.  The only O(B*C) work is the dense
sexp[b] = sum_c exp(30*cos[b,c] - 30); everything else is O(B*LMAX) or
O((B+C)*D) and runs on host in float64.

Device (per core, classes C-sharded 2500/core zero-padded to 2560):
  inputs are pre-normalized, pre-transposed fp8(e4m3, x16) operands
  prepared on host.  Main loop over 16 row-blocks x 5 class-chunks:
  fp8 DoubleRow matmuls (K=256/instr) accumulate dots into PSUM, and
  the ScalarE reads each PSUM bank directly with one Exp activation
  (scale 30/256, bias -30) whose accum_out produces the per-row
  partial sum.  Output is just sexp [128, 80] per core.

Host (numpy, float64): row norms of f and w, normalization + transpose
+ fp8 quantization of the matmul operands, exact positive-class cos
via gather, arcface margin, denominator correction (dedup'd), ragged
CE, mean.  No collectives (cross-core reduction of [2048] scalars
happens on host during unsharding).
"""

import math
import sys

import numpy as np
from ml_dtypes import float8_e4m3

for _p in ("/opt/trn_rl_repo",):
    if _p not in sys.path:
        sys.path.append(_p)

import concourse.bass as bass
import concourse.bacc as bacc
import concourse.mybir as mybir
import concourse.tile as tile
from concourse.bass_utils import run_bass_kernel_spmd
from contextlib import ExitStack

# ---- custom DVE op: fused exp-approx + row-sum ---------------------------
# exp(t) for t = (30/256)*dot - 30 as e^-30 * q(w)^16 with w = dot*C0 and
# q(w) = w^2 + C1*w + C2 a monic quadratic (log-domain weighted fit on the
# actual dot distribution; end-to-end loss error is fp8-dominated).  One
# 8-stage DVE pass per PSUM tile produces the per-row partial sums via the
# lane accumulator, replacing an ACT exp + DVE reduce pair.
import concourse.dve_ops as dve_ops
import concourse.dve_spec as dve_spec
from concourse.dve_spec import AluOp as _DAlu, Bin as _DBin, Spec as _DSpec
from concourse.dve_spec import Src0 as _DSrc0, C0 as _DC0, C1 as _DC1, C2 as _DC2
from concourse.dve_spec import sq as _dsq
from concourse.dve_uop import DveOpSpec as _DveOpSpec
from operator import add as _op_add

EAS_C0 = 0.01176986
EAS_B = 1.22731124
EAS_G = 0.99729751
EAS_P = 8


def _ref_exp16_sum(in0, in1, c0, c1, c2):
    x = in0.astype(np.float32)
    wv = x * np.float32(c0)
    q = (wv + np.float32(c1)) * wv + np.float32(c2)
    out = q ** EAS_P
    acc = out.reshape(out.shape[0], -1).sum(axis=-1, keepdims=True)
    return out.astype(np.float32), acc.astype(np.float32)


def _register_exp16_sum():
    name = "EXP16_SUM_ANT"
    for op in dve_ops.OPS:
        if op.name == name:
            return op
    _w = _DBin(_DAlu.MULTIPLY, _DSrc0, _DC0)
    _q = _DBin(_DAlu.ADD, _DBin(_DAlu.MULTIPLY, _DBin(_DAlu.ADD, _w, _DC1), _w), _DC2)
    body = _dsq(_dsq(_dsq(_q)))
    spec = _DSpec(
        body=body, accum=_op_add, accum_init=dve_spec.Zero,
        reference=_ref_exp16_sum,
    )
    opcode = dve_ops._CUSTOM_DVE_ROW_BASE + len(dve_ops.OPS)
    assert opcode < 0x20
    op = dve_ops.DveOp(name, spec, subdim=False, uops_sha={})
    dve_ops._SUB_OPCODE_FOR_NAME[name] = opcode
    dve_ops.OPS.append(op)
    dve_ops.CUSTOM_DVE_SPECS[name] = spec
    for ver in ("v3",):
        uops = dve_spec.lower(spec, ver=ver)
        op.uops_sha[ver] = _DveOpSpec(
            name=name, opcode=opcode, uops=uops, rd1_en=False
        ).sha(ver)
    return op


EXP16_SUM = _register_exp16_sum()

B, C, D, LMAX = 2048, 20000, 512, 8
NCORES = 8
CSH = C // NCORES          # 2500 real classes per core
CSHP = 2560                # padded to 5*512 (bank-aligned chunks)
NBLK = B // 128            # 16 row blocks
NW = 512                   # matmul N-chunk width (one fp32 PSUM bank)
NCH = CSHP // NW           # 5 chunks per block per core
KC = D // 128              # 4 contraction chunks
NFP = 4                    # f pieces / supergroups (4 row-blocks each)
# clean-tile consumers: 11 blocks on ACT, 5 on the fused DVE op
ACT_BLOCKS = frozenset((0, 1, 2, 4, 5, 6, 8, 9, 10, 12, 13))
S = 30.0
M_MARGIN = 0.5

F32 = mybir.dt.float32
BF16 = mybir.dt.bfloat16
FP8 = mybir.dt.float8e4
F8S = 16.0                 # fp8 pre-scale per operand (dots carry 256x)

_GRAPH = None


def build_graph():
    nc = bacc.Bacc()
    ft_ext = [
        nc.declare_dram_parameter(f"ft8_{q}", [128, KC, NW], FP8, isOutput=False)
        for q in range(NFP)
    ]
    wt_ext = [
        nc.declare_dram_parameter(f"wt8_{n}", [128, KC, NW], FP8, isOutput=False)
        for n in range(NCH)
    ]
    parts_ext = nc.declare_dram_parameter("parts", [128, 2 * NBLK], F32, isOutput=True)

    AF = mybir.ActivationFunctionType

    with ExitStack() as ctx:
        tc = ctx.enter_context(tile.TileContext(nc))
        const = ctx.enter_context(tc.tile_pool(name="const", bufs=1))
        resident = ctx.enter_context(tc.tile_pool(name="resident", bufs=1))
        esp = ctx.enter_context(tc.tile_pool(name="esp", bufs=3))
        dummy = ctx.enter_context(tc.tile_pool(name="dummy", bufs=3))
        pmm = ctx.enter_context(tc.tile_pool(name="pmm", bufs=2, space="PSUM"))

        nbias = const.tile([128, 1], F32)
        nc.vector.memset(nbias[:], -S)

        fT = [resident.tile([128, KC, NW], FP8, name=f"fT{q}") for q in range(NFP)]
        wT = [resident.tile([128, KC, NW], FP8, name=f"wT{n}") for n in range(NCH)]
        # level-1 partial sums: one bf16 value per 128-element quarter-chunk
        parts_t = resident.tile([128, 2 * NBLK], F32)

        # spread input DMA descriptor writes over all engine queues, in the
        # order the matmul stream consumes the pieces; the first f/w pieces
        # are split into k-halves so the first matmul can start sooner
        loads = [
            (fT[0][:, 0:2, :], ft_ext[0][:, 0:2, :]),
            (wT[0][:, 0:2, :], wt_ext[0][:, 0:2, :]),
            (fT[0][:, 2:4, :], ft_ext[0][:, 2:4, :]),
            (wT[0][:, 2:4, :], wt_ext[0][:, 2:4, :]),
            (wT[4][:], wt_ext[4][:, :, :]),
            (wT[1][:], wt_ext[1][:, :, :]),
            (wT[2][:], wt_ext[2][:, :, :]),
            (wT[3][:], wt_ext[3][:, :, :]),
            (fT[1][:], ft_ext[1][:, :, :]),
            (fT[2][:], ft_ext[2][:, :, :]),
            (fT[3][:], ft_ext[3][:, :, :]),
        ]
        queues = [nc.sync, nc.scalar, nc.gpsimd]
        for i, (dst, src) in enumerate(loads):
            queues[i % len(queues)].dma_start(dst, src)

        # Chunk stream per supergroup g (= f piece, 4 row blocks): four
        # block-aligned "clean" tiles (chunks 0-3 of one block) then one
        # "c4" tile (chunk 4 of each of the 4 blocks).  Clean tiles are
        # consumed either by ACT (Exp + accum_out) or by the fused DVE
        # EXP16_SUM op; c4 tiles always by DVE (4 sub-slices, one accum
        # column per block).  parts col b = clean sum, col 16+b = c4 sum.
        def clean_tile(g, i):
            b = NFP * g + i
            ps = pmm.tile([128, 4, NW], F32, tag="mm", name=f"ps_c{b}")
            for n in range(4):
                for k2 in range(KC // 2):
                    nc.tensor.matmul(
                        ps[:, n, :],
                        fT[g][:, 2 * k2 : 2 * k2 + 2, i * 128 : (i + 1) * 128],
                        wT[n][:, 2 * k2 : 2 * k2 + 2, :],
                        start=(k2 == 0),
                        stop=(k2 == KC // 2 - 1),
                        perf_mode=mybir.MatmulPerfMode.DoubleRow,
                    )
            if b in ACT_BLOCKS:
                ed = esp.tile([128, 4 * NW], BF16, tag="ed", name=f"ed_{b}")
                nc.scalar.activation(
                    ed[:], ps[:, :, :], AF.Exp,
                    bias=nbias[:], scale=S / (F8S * F8S),
                    accum_out=parts_t[:, b : b + 1],
                )
            else:
                dm = dummy.tile([128, 4 * NW], BF16, tag="dm", name=f"dm_{b}")
                nc.vector._custom_dve(
                    EXP16_SUM, out=dm[:], in0=ps[:, :, :],
                    s0=EAS_C0, s1=EAS_B, imm2=EAS_G,
                    accum_out=parts_t[:, b : b + 1],
                )

        def nasty_tile(g):
            psn = pmm.tile([128, 4, NW], F32, tag="mm", name=f"ps_n{g}")
            for i in range(4):
                for k2 in range(KC // 2):
                    nc.tensor.matmul(
                        psn[:, i, :],
                        fT[g][:, 2 * k2 : 2 * k2 + 2, i * 128 : (i + 1) * 128],
                        wT[4][:, 2 * k2 : 2 * k2 + 2, :],
                        start=(k2 == 0),
                        stop=(k2 == KC // 2 - 1),
                        perf_mode=mybir.MatmulPerfMode.DoubleRow,
                    )
            dmn = dummy.tile([128, 4, NW], BF16, tag="dmn", name=f"dmn_{g}")
            for i in range(4):
                b = NFP * g + i
                nc.vector._custom_dve(
                    EXP16_SUM, out=dmn[:, i, :], in0=psn[:, i, :],
                    s0=EAS_C0, s1=EAS_B, imm2=EAS_G,
                    accum_out=parts_t[:, NBLK + b : NBLK + b + 1],
                )

        # interleave ACT- and DVE-consumed tiles so neither engine gets a
        # back-to-back run that stalls the 2-deep PSUM ping-pong
        for g in range(NFP):
            if g < 3:
                order = [(0,), None, (1,), (3,), (2,)]   # A D A D A
            else:
                order = [(0,), None, (1,), (2,), (3,)]   # A D A D D
            for item in order:
                if item is None:
                    nasty_tile(g)
                else:
                    clean_tile(g, item[0])
            if g == 1:
                nc.gpsimd.dma_start(parts_ext[:, 0:8], parts_t[:, 0:8])
                nc.gpsimd.dma_start(
                    parts_ext[:, NBLK : NBLK + 8], parts_t[:, NBLK : NBLK + 8]
                )
        nc.sync.dma_start(parts_ext[:, 8:NBLK], parts_t[:, 8:NBLK])
        nc.sync.dma_start(parts_ext[:, NBLK + 8 :], parts_t[:, NBLK + 8 :])

    nc.finalize()
    return nc


def _get_graph():
    global _GRAPH
    if _GRAPH is None:
        _GRAPH = build_graph()
    return _GRAPH


def _to_kpn(xT):
    """[D, N] (d-major) -> [128, KC, N] with partition p = d % 128, k = d // 128."""
    Dd, N = xT.shape
    return np.ascontiguousarray(xT.reshape(KC, 128, N).transpose(1, 0, 2))


def make_in_maps(f, lab_word2vec, lab_pinds=None):
    f = np.asarray(f, dtype=np.float64)
    w = np.asarray(lab_word2vec, dtype=np.float64)
    fn = np.linalg.norm(f, axis=1)
    wn = np.linalg.norm(w, axis=1)
    fhatT = (F8S * (f / fn[:, None]).T).astype(np.float32)   # [D, B]
    ft_kpn = _to_kpn(fhatT).astype(float8_e4m3)              # [128, KC, B]
    ft_pieces = [
        np.ascontiguousarray(ft_kpn[:, :, q * NW : (q + 1) * NW]) for q in range(NFP)
    ]
    in_maps = []
    for i in range(NCORES):
        wpad = np.zeros((CSHP, D), dtype=np.float64)
        wsh = w[i * CSH : (i + 1) * CSH]
        wpad[:CSH] = wsh / wn[i * CSH : (i + 1) * CSH, None]
        wt_kpn = _to_kpn((F8S * wpad.T).astype(np.float32)).astype(float8_e4m3)
        m = {f"ft8_{q}": ft_pieces[q] for q in range(NFP)}
        for n in range(NCH):
            m[f"wt8_{n}"] = np.ascontiguousarray(wt_kpn[:, :, n * NW : (n + 1) * NW])
        in_maps.append(m)
    return in_maps


def host_finish(outs, f, lab_word2vec, lab_pinds, lengths):
    """outs: list of 8 dicts with sexp. Returns float32 loss."""
    f = np.asarray(f, dtype=np.float64)
    w = np.asarray(lab_word2vec, dtype=np.float64)
    pinds = np.asarray(lab_pinds, dtype=np.int64)
    lens = np.asarray(lengths, dtype=np.int64)

    # S_shift[b] = sum_c exp(30 cos - 30).  parts col b = clean sum (exp on
    # ACT for ACT_BLOCKS, else q^16 from the DVE op, scaled by e^-30 here);
    # col 16+b = c4 sum (always DVE).  The 60 zero-pad classes sit in the c4
    # chunk and contribute q(0)^16 * e^-30 each (dot exactly 0).
    k_eas = math.exp(-S)
    pad_term = (CSHP - CSH) * (EAS_G ** EAS_P) * k_eas
    s_shift = np.zeros(B, dtype=np.float64)
    for i in range(NCORES):
        pa = outs[i]["parts"].astype(np.float64)         # [128, 2*NBLK]
        per_block = np.empty((128, NBLK), dtype=np.float64)
        for b in range(NBLK):
            clean = pa[:, b] if b in ACT_BLOCKS else pa[:, b] * k_eas
            per_block[:, b] = clean + pa[:, NBLK + b] * k_eas - pad_term
        s_shift += per_block.T.reshape(B)                # b = m*128 + p

    # exact positive-class cosines on host
    fn = np.linalg.norm(f, axis=1)                       # [B]
    wn = np.linalg.norm(w, axis=1)                       # [C]
    wsel = w[pinds]                                      # [B, LMAX, D]
    dots = np.einsum("bd,bld->bl", f, wsel)              # [B, LMAX]
    cos = dots / np.maximum(fn[:, None] * wn[pinds], 1e-8)

    cos_m, sin_m = math.cos(M_MARGIN), math.sin(M_MARGIN)
    th = math.cos(math.pi - M_MARGIN)
    mm = math.sin(math.pi - M_MARGIN) * M_MARGIN
    sine = np.sqrt(np.clip(1.0 - cos * cos, 0.0, 1.0))
    phi = cos * cos_m - sine * sin_m
    phi = np.where(cos > th, phi, cos - mm)

    mask = (np.arange(LMAX)[None, :] < lens[:, None])    # [B, LMAX] bool
    # dedup: a class replaced once in the denominator even if in 2 slots
    dup = np.zeros_like(mask)
    for j in range(1, LMAX):
        for j2 in range(j):
            dup[:, j] |= mask[:, j2] & (pinds[:, j2] == pinds[:, j])
    uniq = mask & ~dup
    corr = (uniq * (np.exp(S * phi - S) - np.exp(S * cos - S))).sum(axis=1)
    z = S + np.log(s_shift + corr)                       # logsumexp, [B]
    pos_sum = (mask * (S * phi)).sum(axis=1)
    L = lens.astype(np.float64)
    per_sample = (L * z - pos_sum) / (L * L)
    return np.float32(per_sample.mean())


def kernel(f, labels, lab_word2vec, lab_pinds, lengths):
    nc = _get_graph()
    in_maps = make_in_maps(f, lab_word2vec)
    res = run_bass_kernel_spmd(nc, in_maps, core_ids=list(range(NCORES)))
    return host_finish(res.results, f, lab_word2vec, lab_pinds, lengths)


# revision 43
# speedup vs baseline: 1.3491x; 1.3491x over previous
"""ArcFace combined-margin loss kernel for 8 TRN2 NeuronCores.

Strategy
--------
reference: cos = (f @ w.T) / (|f||w|); phi = arcface(cos);
outputs = s*(labels*phi + (1-labels)*cos); loss = mean over rows of
-(sum of log_softmax(outputs) at lab_pinds, masked) / L^2.

labels is the multi-hot of (lab_pinds, lengths), so outputs differs from
s*cos only at <=8 entries/row.  The only O(B*C) work is the dense
sexp[b] = sum_c exp(30*cos[b,c] - 30); everything else is O(B*LMAX) or
O((B+C)*D) and runs on host in float64.

Device (per core, classes C-sharded 2500/core zero-padded to 2560):
  inputs are pre-normalized, pre-transposed fp8(e4m3, x16) operands
  prepared on host.  Main loop over 16 row-blocks x 5 class-chunks:
  fp8 DoubleRow matmuls (K=256/instr) accumulate dots into PSUM, and
  the ScalarE reads each PSUM bank directly with one Exp activation
  (scale 30/256, bias -30) whose accum_out produces the per-row
  partial sum.  Output is just sexp [128, 80] per core.

Host (numpy, float64): row norms of f and w, normalization + transpose
+ fp8 quantization of the matmul operands, exact positive-class cos
via gather, arcface margin, denominator correction (dedup'd), ragged
CE, mean.  No collectives (cross-core reduction of [2048] scalars
happens on host during unsharding).
"""

import math
import sys

import numpy as np
from ml_dtypes import float8_e4m3

for _p in ("/opt/trn_rl_repo",):
    if _p not in sys.path:
        sys.path.append(_p)

import concourse.bass as bass
import concourse.bacc as bacc
import concourse.mybir as mybir
import concourse.tile as tile
from concourse.bass_utils import run_bass_kernel_spmd
from contextlib import ExitStack

# ---- custom DVE op: fused exp-approx + row-sum ---------------------------
# exp(t) for t = (30/256)*dot - 30 as e^-30 * q(w)^16 with w = dot*C0 and
# q(w) = w^2 + C1*w + C2 a monic quadratic (log-domain weighted fit on the
# actual dot distribution; end-to-end loss error is fp8-dominated).  One
# 8-stage DVE pass per PSUM tile produces the per-row partial sums via the
# lane accumulator, replacing an ACT exp + DVE reduce pair.
import concourse.dve_ops as dve_ops
import concourse.dve_spec as dve_spec
from concourse.dve_spec import AluOp as _DAlu, Bin as _DBin, Spec as _DSpec
from concourse.dve_spec import Src0 as _DSrc0, C0 as _DC0, C1 as _DC1, C2 as _DC2
from concourse.dve_spec import sq as _dsq
from concourse.dve_uop import DveOpSpec as _DveOpSpec
from operator import add as _op_add

EAS_C0 = 0.01176986
EAS_B = 1.22731124
EAS_G = 0.99729751
EAS_P = 8


def _ref_exp16_sum(in0, in1, c0, c1, c2):
    x = in0.astype(np.float32)
    wv = x * np.float32(c0)
    q = (wv + np.float32(c1)) * wv + np.float32(c2)
    out = q ** EAS_P
    acc = out.reshape(out.shape[0], -1).sum(axis=-1, keepdims=True)
    return out.astype(np.float32), acc.astype(np.float32)


def _register_exp16_sum():
    name = "EXP16_SUM_ANT"
    for op in dve_ops.OPS:
        if op.name == name:
            return op
    _w = _DBin(_DAlu.MULTIPLY, _DSrc0, _DC0)
    _q = _DBin(_DAlu.ADD, _DBin(_DAlu.MULTIPLY, _DBin(_DAlu.ADD, _w, _DC1), _w), _DC2)
    body = _dsq(_dsq(_dsq(_q)))
    spec = _DSpec(
        body=body, accum=_op_add, accum_init=dve_spec.Zero,
        reference=_ref_exp16_sum,
    )
    opcode = dve_ops._CUSTOM_DVE_ROW_BASE + len(dve_ops.OPS)
    assert opcode < 0x20
    op = dve_ops.DveOp(name, spec, subdim=False, uops_sha={})
    dve_ops._SUB_OPCODE_FOR_NAME[name] = opcode
    dve_ops.OPS.append(op)
    dve_ops.CUSTOM_DVE_SPECS[name] = spec
    for ver in ("v3",):
        uops = dve_spec.lower(spec, ver=ver)
        op.uops_sha[ver] = _DveOpSpec(
            name=name, opcode=opcode, uops=uops, rd1_en=False
        ).sha(ver)
    return op


EXP16_SUM = _register_exp16_sum()

B, C, D, LMAX = 2048, 20000, 512, 8
NCORES = 8
CSH = C // NCORES          # 2500 real classes per core
CSHP = 2560                # padded to 5*512 (bank-aligned chunks)
NBLK = B // 128            # 16 row blocks
NW = 512                   # matmul N-chunk width (one fp32 PSUM bank)
NCH = CSHP // NW           # 5 chunks per block per core
KC = D // 128              # 4 contraction chunks
NFP = 4                    # f pieces / supergroups (4 row-blocks each)
# clean-tile consumers: 10 blocks on ACT, 6 on the fused DVE op
ACT_BLOCKS = frozenset((0, 1, 2, 4, 5, 6, 8, 9, 12, 13))
S = 30.0
M_MARGIN = 0.5

F32 = mybir.dt.float32
BF16 = mybir.dt.bfloat16
FP8 = mybir.dt.float8e4
F8S = 16.0                 # fp8 pre-scale per operand (dots carry 256x)

_GRAPH = None


def build_graph():
    nc = bacc.Bacc()
    ft_ext = [
        nc.declare_dram_parameter(f"ft8_{q}", [128, KC, NW], FP8, isOutput=False)
        for q in range(NFP)
    ]
    wt_ext = [
        nc.declare_dram_parameter(f"wt8_{n}", [128, KC, NW], FP8, isOutput=False)
        for n in range(NCH)
    ]
    parts_ext = nc.declare_dram_parameter("parts", [128, 3 * NBLK], F32, isOutput=True)

    AF = mybir.ActivationFunctionType

    with ExitStack() as ctx:
        tc = ctx.enter_context(tile.TileContext(nc))
        const = ctx.enter_context(tc.tile_pool(name="const", bufs=1))
        resident = ctx.enter_context(tc.tile_pool(name="resident", bufs=1))
        esp = ctx.enter_context(tc.tile_pool(name="esp", bufs=3))
        dummy = ctx.enter_context(tc.tile_pool(name="dummy", bufs=3))
        pmm = ctx.enter_context(tc.tile_pool(name="pmm", bufs=4, space="PSUM"))

        nbias = const.tile([128, 1], F32)
        nc.vector.memset(nbias[:], -S)

        fT = [resident.tile([128, KC, NW], FP8, name=f"fT{q}") for q in range(NFP)]
        wT = [resident.tile([128, KC, NW], FP8, name=f"wT{n}") for n in range(NCH)]
        # level-1 partial sums: one bf16 value per 128-element quarter-chunk
        parts_t = resident.tile([128, 3 * NBLK], F32)

        # spread input DMA descriptor writes over all engine queues, in the
        # order the matmul stream consumes the pieces; the first f/w pieces
        # are split into k-halves so the first matmul can start sooner
        loads = [
            (fT[0][:, 0:2, :], ft_ext[0][:, 0:2, :]),
            (wT[0][:, 0:2, :], wt_ext[0][:, 0:2, :]),
            (fT[0][:, 2:4, :], ft_ext[0][:, 2:4, :]),
            (wT[0][:, 2:4, :], wt_ext[0][:, 2:4, :]),
            (wT[4][:], wt_ext[4][:, :, :]),
            (wT[1][:], wt_ext[1][:, :, :]),
            (wT[2][:], wt_ext[2][:, :, :]),
            (wT[3][:], wt_ext[3][:, :, :]),
            (fT[1][:], ft_ext[1][:, :, :]),
            (fT[2][:], ft_ext[2][:, :, :]),
            (fT[3][:], ft_ext[3][:, :, :]),
        ]
        queues = [nc.sync, nc.scalar, nc.gpsimd]
        for i, (dst, src) in enumerate(loads):
            queues[i % len(queues)].dma_start(dst, src)

        # Chunk stream per supergroup g (= f piece, 4 row blocks): four
        # block-aligned "clean" tiles (chunks 0-3 of one block) then one
        # "c4" tile (chunk 4 of each of the 4 blocks).  Clean tiles are
        # consumed either by ACT (Exp + accum_out) or by the fused DVE
        # EXP16_SUM op; c4 tiles always by DVE (4 sub-slices, one accum
        # column per block).  parts col b = clean sum, col 16+b = c4 sum.
        # half-tiles: 2 PSUM banks = 2 chunks, 4-deep ping-pong.  Clean half
        # (b, h) = chunks 2h, 2h+1 of block b -> one accum col 2b+h; nasty
        # half (g, nh) = c4 chunks of blocks 4g+2nh, 4g+2nh+1 -> one accum
        # col 32+b per block.
        def clean_half(g, i, h):
            b = NFP * g + i
            ps = pmm.tile([128, 2, NW], F32, tag="mm", name=f"ps_c{b}_{h}")
            for n in (2 * h, 2 * h + 1):
                for k2 in range(KC // 2):
                    nc.tensor.matmul(
                        ps[:, n - 2 * h, :],
                        fT[g][:, 2 * k2 : 2 * k2 + 2, i * 128 : (i + 1) * 128],
                        wT[n][:, 2 * k2 : 2 * k2 + 2, :],
                        start=(k2 == 0),
                        stop=(k2 == KC // 2 - 1),
                        perf_mode=mybir.MatmulPerfMode.DoubleRow,
                    )
            col = 2 * b + h
            if b in ACT_BLOCKS:
                ed = esp.tile([128, 2 * NW], BF16, tag="ed", name=f"ed_{b}_{h}")
                nc.scalar.activation(
                    ed[:], ps[:, :, :], AF.Exp,
                    bias=nbias[:], scale=S / (F8S * F8S),
                    accum_out=parts_t[:, col : col + 1],
                )
            else:
                dm = dummy.tile([128, 2 * NW], BF16, tag="dm", name=f"dm_{b}_{h}")
                nc.vector._custom_dve(
                    EXP16_SUM, out=dm[:], in0=ps[:, :, :],
                    s0=EAS_C0, s1=EAS_B, imm2=EAS_G,
                    accum_out=parts_t[:, col : col + 1],
                )

        def nasty_half(g, nh):
            ps = pmm.tile([128, 2, NW], F32, tag="mm", name=f"ps_n{g}_{nh}")
            for ii in range(2):
                i = 2 * nh + ii
                for k2 in range(KC // 2):
                    nc.tensor.matmul(
                        ps[:, ii, :],
                        fT[g][:, 2 * k2 : 2 * k2 + 2, i * 128 : (i + 1) * 128],
                        wT[4][:, 2 * k2 : 2 * k2 + 2, :],
                        start=(k2 == 0),
                        stop=(k2 == KC // 2 - 1),
                        perf_mode=mybir.MatmulPerfMode.DoubleRow,
                    )
            dmn = dummy.tile([128, 2, NW], BF16, tag="dmn", name=f"dmn_{g}_{nh}")
            for ii in range(2):
                b = NFP * g + 2 * nh + ii
                nc.vector._custom_dve(
                    EXP16_SUM, out=dmn[:, ii, :], in0=ps[:, ii, :],
                    s0=EAS_C0, s1=EAS_B, imm2=EAS_G,
                    accum_out=parts_t[:, 2 * NBLK + b : 2 * NBLK + b + 1],
                )

        # interleaved half-tile schedules (A=ACT clean, D=DVE clean, N=nasty)
        for g in range(NFP):
            if g < 2:
                a1, a2, a3, dd = 0, 1, 2, 3
                seq = [
                    ("c", a1, 0), ("c", a1, 1), ("c", dd, 0), ("c", a2, 0),
                    ("n", 0), ("c", a2, 1), ("n", 1), ("c", a3, 0),
                    ("c", a3, 1), ("c", dd, 1),
                ]
            else:
                a1, a2, d1, d2 = 0, 1, 2, 3
                seq = [
                    ("c", d1, 0), ("c", d1, 1), ("c", a1, 0), ("c", d2, 0),
                    ("n", 0), ("c", a1, 1), ("c", a2, 0), ("n", 1),
                    ("c", a2, 1), ("c", d2, 1),
                ]
            for item in seq:
                if item[0] == "c":
                    clean_half(g, item[1], item[2])
                else:
                    nasty_half(g, item[1])
            if g == 1:
                nc.gpsimd.dma_start(parts_ext[:, 0:16], parts_t[:, 0:16])
                nc.gpsimd.dma_start(
                    parts_ext[:, 2 * NBLK : 2 * NBLK + 8],
                    parts_t[:, 2 * NBLK : 2 * NBLK + 8],
                )
        nc.sync.dma_start(parts_ext[:, 16 : 2 * NBLK], parts_t[:, 16 : 2 * NBLK])
        nc.sync.dma_start(
            parts_ext[:, 2 * NBLK + 8 :], parts_t[:, 2 * NBLK + 8 :]
        )

    nc.finalize()
    return nc


def _get_graph():
    global _GRAPH
    if _GRAPH is None:
        _GRAPH = build_graph()
    return _GRAPH


def _to_kpn(xT):
    """[D, N] (d-major) -> [128, KC, N] with partition p = d % 128, k = d // 128."""
    Dd, N = xT.shape
    return np.ascontiguousarray(xT.reshape(KC, 128, N).transpose(1, 0, 2))


def make_in_maps(f, lab_word2vec, lab_pinds=None):
    f = np.asarray(f, dtype=np.float64)
    w = np.asarray(lab_word2vec, dtype=np.float64)
    fn = np.linalg.norm(f, axis=1)
    wn = np.linalg.norm(w, axis=1)
    fhatT = (F8S * (f / fn[:, None]).T).astype(np.float32)   # [D, B]
    ft_kpn = _to_kpn(fhatT).astype(float8_e4m3)              # [128, KC, B]
    ft_pieces = [
        np.ascontiguousarray(ft_kpn[:, :, q * NW : (q + 1) * NW]) for q in range(NFP)
    ]
    in_maps = []
    for i in range(NCORES):
        wpad = np.zeros((CSHP, D), dtype=np.float64)
        wsh = w[i * CSH : (i + 1) * CSH]
        wpad[:CSH] = wsh / wn[i * CSH : (i + 1) * CSH, None]
        wt_kpn = _to_kpn((F8S * wpad.T).astype(np.float32)).astype(float8_e4m3)
        m = {f"ft8_{q}": ft_pieces[q] for q in range(NFP)}
        for n in range(NCH):
            m[f"wt8_{n}"] = np.ascontiguousarray(wt_kpn[:, :, n * NW : (n + 1) * NW])
        in_maps.append(m)
    return in_maps


def host_finish(outs, f, lab_word2vec, lab_pinds, lengths):
    """outs: list of 8 dicts with sexp. Returns float32 loss."""
    f = np.asarray(f, dtype=np.float64)
    w = np.asarray(lab_word2vec, dtype=np.float64)
    pinds = np.asarray(lab_pinds, dtype=np.int64)
    lens = np.asarray(lengths, dtype=np.int64)

    # S_shift[b] = sum_c exp(30 cos - 30).  parts col b = clean sum (exp on
    # ACT for ACT_BLOCKS, else q^16 from the DVE op, scaled by e^-30 here);
    # col 16+b = c4 sum (always DVE).  The 60 zero-pad classes sit in the c4
    # chunk and contribute q(0)^16 * e^-30 each (dot exactly 0).
    k_eas = math.exp(-S)
    pad_term = (CSHP - CSH) * (EAS_G ** EAS_P) * k_eas
    s_shift = np.zeros(B, dtype=np.float64)
    for i in range(NCORES):
        pa = outs[i]["parts"].astype(np.float64)         # [128, 3*NBLK]
        per_block = np.empty((128, NBLK), dtype=np.float64)
        for b in range(NBLK):
            clean = pa[:, 2 * b] + pa[:, 2 * b + 1]
            if b not in ACT_BLOCKS:
                clean *= k_eas
            per_block[:, b] = clean + pa[:, 2 * NBLK + b] * k_eas - pad_term
        s_shift += per_block.T.reshape(B)                # b = m*128 + p

    # exact positive-class cosines on host
    fn = np.linalg.norm(f, axis=1)                       # [B]
    wn = np.linalg.norm(w, axis=1)                       # [C]
    wsel = w[pinds]                                      # [B, LMAX, D]
    dots = np.einsum("bd,bld->bl", f, wsel)              # [B, LMAX]
    cos = dots / np.maximum(fn[:, None] * wn[pinds], 1e-8)

    cos_m, sin_m = math.cos(M_MARGIN), math.sin(M_MARGIN)
    th = math.cos(math.pi - M_MARGIN)
    mm = math.sin(math.pi - M_MARGIN) * M_MARGIN
    sine = np.sqrt(np.clip(1.0 - cos * cos, 0.0, 1.0))
    phi = cos * cos_m - sine * sin_m
    phi = np.where(cos > th, phi, cos - mm)

    mask = (np.arange(LMAX)[None, :] < lens[:, None])    # [B, LMAX] bool
    # dedup: a class replaced once in the denominator even if in 2 slots
    dup = np.zeros_like(mask)
    for j in range(1, LMAX):
        for j2 in range(j):
            dup[:, j] |= mask[:, j2] & (pinds[:, j2] == pinds[:, j])
    uniq = mask & ~dup
    corr = (uniq * (np.exp(S * phi - S) - np.exp(S * cos - S))).sum(axis=1)
    z = S + np.log(s_shift + corr)                       # logsumexp, [B]
    pos_sum = (mask * (S * phi)).sum(axis=1)
    L = lens.astype(np.float64)
    per_sample = (L * z - pos_sum) / (L * L)
    return np.float32(per_sample.mean())


def kernel(f, labels, lab_word2vec, lab_pinds, lengths):
    nc = _get_graph()
    in_maps = make_in_maps(f, lab_word2vec)
    res = run_bass_kernel_spmd(nc, in_maps, core_ids=list(range(NCORES)))
    return host_finish(res.results, f, lab_word2vec, lab_pinds, lengths)


# revision 51
# speedup vs baseline: 1.3742x; 1.0186x over previous
"""ArcFace combined-margin loss kernel for 8 TRN2 NeuronCores.

Strategy
--------
reference: cos = (f @ w.T) / (|f||w|); phi = arcface(cos);
outputs = s*(labels*phi + (1-labels)*cos); loss = mean over rows of
-(sum of log_softmax(outputs) at lab_pinds, masked) / L^2.

labels is the multi-hot of (lab_pinds, lengths), so outputs differs from
s*cos only at <=8 entries/row.  The only O(B*C) work is the dense
sexp[b] = sum_c exp(30*cos[b,c] - 30); everything else is O(B*LMAX) or
O((B+C)*D) and runs on host in float64.

Device (per core, classes C-sharded 2500/core zero-padded to 2560):
  inputs are pre-normalized, pre-transposed fp8(e4m3, x16) operands
  prepared on host.  Main loop over 16 row-blocks x 5 class-chunks:
  fp8 DoubleRow matmuls (K=256/instr) accumulate dots into PSUM, and
  the ScalarE reads each PSUM bank directly with one Exp activation
  (scale 30/256, bias -30) whose accum_out produces the per-row
  partial sum.  Output is just sexp [128, 80] per core.

Host (numpy, float64): row norms of f and w, normalization + transpose
+ fp8 quantization of the matmul operands, exact positive-class cos
via gather, arcface margin, denominator correction (dedup'd), ragged
CE, mean.  No collectives (cross-core reduction of [2048] scalars
happens on host during unsharding).
"""

import math
import sys

import numpy as np
from ml_dtypes import float8_e4m3

for _p in ("/opt/trn_rl_repo",):
    if _p not in sys.path:
        sys.path.append(_p)

import concourse.bass as bass
import concourse.bacc as bacc
import concourse.mybir as mybir
import concourse.tile as tile
from concourse.bass_utils import run_bass_kernel_spmd
from contextlib import ExitStack

# ---- custom DVE op: fused exp-approx + row-sum ---------------------------
# exp(t) for t = (30/256)*dot - 30 as e^-30 * q(w)^16 with w = dot*C0 and
# q(w) = w^2 + C1*w + C2 a monic quadratic (log-domain weighted fit on the
# actual dot distribution; end-to-end loss error is fp8-dominated).  One
# 8-stage DVE pass per PSUM tile produces the per-row partial sums via the
# lane accumulator, replacing an ACT exp + DVE reduce pair.
import concourse.dve_ops as dve_ops
import concourse.dve_spec as dve_spec
from concourse.dve_spec import AluOp as _DAlu, Bin as _DBin, Spec as _DSpec
from concourse.dve_spec import Src0 as _DSrc0, C0 as _DC0, C1 as _DC1, C2 as _DC2
from concourse.dve_spec import sq as _dsq
from concourse.dve_uop import DveOpSpec as _DveOpSpec
from operator import add as _op_add

EAS_C0 = 0.01176986
EAS_B = 1.22731124
EAS_G = 0.99729751
EAS_P = 8


def _ref_exp16_sum(in0, in1, c0, c1, c2):
    x = in0.astype(np.float32)
    wv = x * np.float32(c0)
    q = (wv + np.float32(c1)) * wv + np.float32(c2)
    out = q ** EAS_P
    acc = out.reshape(out.shape[0], -1).sum(axis=-1, keepdims=True)
    return out.astype(np.float32), acc.astype(np.float32)


def _register_exp16_sum():
    name = "EXP16_SUM_ANT"
    for op in dve_ops.OPS:
        if op.name == name:
            return op
    _w = _DBin(_DAlu.MULTIPLY, _DSrc0, _DC0)
    _q = _DBin(_DAlu.ADD, _DBin(_DAlu.MULTIPLY, _DBin(_DAlu.ADD, _w, _DC1), _w), _DC2)
    body = _dsq(_dsq(_dsq(_q)))
    spec = _DSpec(
        body=body, accum=_op_add, accum_init=dve_spec.Zero,
        reference=_ref_exp16_sum,
    )
    opcode = dve_ops._CUSTOM_DVE_ROW_BASE + len(dve_ops.OPS)
    assert opcode < 0x20
    op = dve_ops.DveOp(name, spec, subdim=False, uops_sha={})
    dve_ops._SUB_OPCODE_FOR_NAME[name] = opcode
    dve_ops.OPS.append(op)
    dve_ops.CUSTOM_DVE_SPECS[name] = spec
    for ver in ("v3",):
        uops = dve_spec.lower(spec, ver=ver)
        op.uops_sha[ver] = _DveOpSpec(
            name=name, opcode=opcode, uops=uops, rd1_en=False
        ).sha(ver)
    return op


EXP16_SUM = _register_exp16_sum()

B, C, D, LMAX = 2048, 20000, 512, 8
NCORES = 8
CSH = C // NCORES          # 2500 real classes per core
NW4 = 464                  # c4 chunk width (2048+464 = 2512, 12 pad classes)
CSHP = 2048 + NW4          # padded class shard
NBLK = B // 128            # 16 row blocks
NW = 512                   # matmul N-chunk width (one fp32 PSUM bank)
NCH = 5                    # 4 full chunks + the narrow c4 chunk
KC = D // 128              # 4 contraction chunks
NFP = 4                    # f pieces / supergroups (4 row-blocks each)
# clean-tile consumers: 10 blocks on ACT, 6 on the fused DVE op
ACT_BLOCKS = frozenset((0, 1, 2, 4, 5, 6, 8, 9, 12, 13))
S = 30.0
M_MARGIN = 0.5

F32 = mybir.dt.float32
BF16 = mybir.dt.bfloat16
FP8 = mybir.dt.float8e4
F8S = 16.0                 # fp8 pre-scale per operand (dots carry 256x)

_GRAPH = None


def build_graph():
    nc = bacc.Bacc()
    ft_ext = [
        nc.declare_dram_parameter(f"ft8_{q}", [128, KC, NW], FP8, isOutput=False)
        for q in range(NFP)
    ]
    wt_ext = [
        nc.declare_dram_parameter(
            f"wt8_{n}", [128, KC, NW if n < 4 else NW4], FP8, isOutput=False
        )
        for n in range(NCH)
    ]
    parts_ext = nc.declare_dram_parameter("parts", [128, 3 * NBLK], F32, isOutput=True)

    AF = mybir.ActivationFunctionType

    with ExitStack() as ctx:
        tc = ctx.enter_context(tile.TileContext(nc))
        const = ctx.enter_context(tc.tile_pool(name="const", bufs=1))
        resident = ctx.enter_context(tc.tile_pool(name="resident", bufs=1))
        esp = ctx.enter_context(tc.tile_pool(name="esp", bufs=3))
        dummy = ctx.enter_context(tc.tile_pool(name="dummy", bufs=3))
        pmm = ctx.enter_context(tc.tile_pool(name="pmm", bufs=4, space="PSUM"))

        nbias = const.tile([128, 1], F32)
        nc.vector.memset(nbias[:], -S)

        fT = [resident.tile([128, KC, NW], FP8, name=f"fT{q}") for q in range(NFP)]
        wT = [
            resident.tile([128, KC, NW if n < 4 else NW4], FP8, name=f"wT{n}")
            for n in range(NCH)
        ]
        # level-1 partial sums: one bf16 value per 128-element quarter-chunk
        parts_t = resident.tile([128, 3 * NBLK], F32)

        # spread input DMA descriptor writes over all engine queues, in the
        # order the matmul stream consumes the pieces; the first f/w pieces
        # are split into k-halves so the first matmul can start sooner
        loads = [
            (wT[0][:, 0:2, :], wt_ext[0][:, 0:2, :]),
            (fT[0][:, 0:2, :], ft_ext[0][:, 0:2, :]),
            (wT[1][:, 0:2, :], wt_ext[1][:, 0:2, :]),
            (wT[0][:, 2:4, :], wt_ext[0][:, 2:4, :]),
            (fT[0][:, 2:4, :], ft_ext[0][:, 2:4, :]),
            (wT[1][:, 2:4, :], wt_ext[1][:, 2:4, :]),
            (wT[2][:], wt_ext[2][:, :, :]),
            (wT[3][:], wt_ext[3][:, :, :]),
            (wT[4][:], wt_ext[4][:, :, :]),
            (fT[1][:], ft_ext[1][:, :, :]),
            (fT[2][:], ft_ext[2][:, :, :]),
            (fT[3][:], ft_ext[3][:, :, :]),
        ]
        queues = [nc.sync, nc.scalar, nc.gpsimd]
        for i, (dst, src) in enumerate(loads):
            queues[i % len(queues)].dma_start(dst, src)

        # Chunk stream per supergroup g (= f piece, 4 row blocks): four
        # block-aligned "clean" tiles (chunks 0-3 of one block) then one
        # "c4" tile (chunk 4 of each of the 4 blocks).  Clean tiles are
        # consumed either by ACT (Exp + accum_out) or by the fused DVE
        # EXP16_SUM op; c4 tiles always by DVE (4 sub-slices, one accum
        # column per block).  parts col b = clean sum, col 16+b = c4 sum.
        # half-tiles: 2 PSUM banks = 2 chunks, 4-deep ping-pong.  Clean half
        # (b, h) = chunks 2h, 2h+1 of block b -> one accum col 2b+h; nasty
        # half (g, nh) = c4 chunks of blocks 4g+2nh, 4g+2nh+1 -> one accum
        # col 32+b per block.
        def clean_half(g, i, h):
            b = NFP * g + i
            ps = pmm.tile([128, 2, NW], F32, tag="mm", name=f"ps_c{b}_{h}")
            for n in (2 * h, 2 * h + 1):
                for k2 in range(KC // 2):
                    nc.tensor.matmul(
                        ps[:, n - 2 * h, :],
                        fT[g][:, 2 * k2 : 2 * k2 + 2, i * 128 : (i + 1) * 128],
                        wT[n][:, 2 * k2 : 2 * k2 + 2, :],
                        start=(k2 == 0),
                        stop=(k2 == KC // 2 - 1),
                        perf_mode=mybir.MatmulPerfMode.DoubleRow,
                    )
            col = 2 * b + h
            if b in ACT_BLOCKS:
                ed = esp.tile([128, 2 * NW], BF16, tag="ed", name=f"ed_{b}_{h}")
                nc.scalar.activation(
                    ed[:], ps[:, :, :], AF.Exp,
                    bias=nbias[:], scale=S / (F8S * F8S),
                    accum_out=parts_t[:, col : col + 1],
                )
            else:
                dm = dummy.tile([128, 2 * NW], BF16, tag="dm", name=f"dm_{b}_{h}")
                nc.vector._custom_dve(
                    EXP16_SUM, out=dm[:], in0=ps[:, :, :],
                    s0=EAS_C0, s1=EAS_B, imm2=EAS_G,
                    accum_out=parts_t[:, col : col + 1],
                )

        def nasty_half(g, nh):
            ps = pmm.tile([128, 2, NW], F32, tag="mm", name=f"ps_n{g}_{nh}")
            for ii in range(2):
                i = 2 * nh + ii
                for k2 in range(KC // 2):
                    nc.tensor.matmul(
                        ps[:, ii, 0:NW4],
                        fT[g][:, 2 * k2 : 2 * k2 + 2, i * 128 : (i + 1) * 128],
                        wT[4][:, 2 * k2 : 2 * k2 + 2, :],
                        start=(k2 == 0),
                        stop=(k2 == KC // 2 - 1),
                        perf_mode=mybir.MatmulPerfMode.DoubleRow,
                    )
            dmn = dummy.tile([128, 2, NW], BF16, tag="dmn", name=f"dmn_{g}_{nh}")
            for ii in range(2):
                b = NFP * g + 2 * nh + ii
                nc.vector._custom_dve(
                    EXP16_SUM, out=dmn[:, ii, 0:NW4], in0=ps[:, ii, 0:NW4],
                    s0=EAS_C0, s1=EAS_B, imm2=EAS_G,
                    accum_out=parts_t[:, 2 * NBLK + b : 2 * NBLK + b + 1],
                )

        # interleaved half-tile schedules (A=ACT clean, D=DVE clean, N=nasty)
        for g in range(NFP):
            if g < 2:
                a1, a2, a3, dd = 0, 1, 2, 3
                seq = [
                    ("c", a1, 0), ("c", a1, 1), ("c", dd, 0), ("c", a2, 0),
                    ("n", 0), ("c", a2, 1), ("n", 1), ("c", a3, 0),
                    ("c", a3, 1), ("c", dd, 1),
                ]
            elif g == 2:
                a1, a2, d1, d2 = 0, 1, 2, 3
                seq = [
                    ("c", d1, 0), ("c", d1, 1), ("c", a1, 0), ("c", d2, 0),
                    ("n", 0), ("c", a1, 1), ("c", a2, 0), ("n", 1),
                    ("c", a2, 1), ("c", d2, 1),
                ]
            else:
                # end the stream on ACT consumers (shorter drain tail)
                a1, a2, d1, d2 = 0, 1, 2, 3
                seq = [
                    ("c", d1, 0), ("c", d1, 1), ("c", a1, 0), ("c", d2, 0),
                    ("n", 0), ("c", a1, 1), ("c", d2, 1), ("n", 1),
                    ("c", a2, 0), ("c", a2, 1),
                ]
            for item in seq:
                if item[0] == "c":
                    clean_half(g, item[1], item[2])
                else:
                    nasty_half(g, item[1])
            if g == 1:
                nc.gpsimd.dma_start(parts_ext[:, 0:16], parts_t[:, 0:16])
                nc.gpsimd.dma_start(
                    parts_ext[:, 2 * NBLK : 2 * NBLK + 8],
                    parts_t[:, 2 * NBLK : 2 * NBLK + 8],
                )
        nc.sync.dma_start(parts_ext[:, 16 : 2 * NBLK], parts_t[:, 16 : 2 * NBLK])
        nc.sync.dma_start(
            parts_ext[:, 2 * NBLK + 8 :], parts_t[:, 2 * NBLK + 8 :]
        )

    nc.finalize()
    return nc


def _get_graph():
    global _GRAPH
    if _GRAPH is None:
        _GRAPH = build_graph()
    return _GRAPH


def _to_kpn(xT):
    """[D, N] (d-major) -> [128, KC, N] with partition p = d % 128, k = d // 128."""
    Dd, N = xT.shape
    return np.ascontiguousarray(xT.reshape(KC, 128, N).transpose(1, 0, 2))


def make_in_maps(f, lab_word2vec, lab_pinds=None):
    f = np.asarray(f, dtype=np.float64)
    w = np.asarray(lab_word2vec, dtype=np.float64)
    fn = np.linalg.norm(f, axis=1)
    wn = np.linalg.norm(w, axis=1)
    fhatT = (F8S * (f / fn[:, None]).T).astype(np.float32)   # [D, B]
    ft_kpn = _to_kpn(fhatT).astype(float8_e4m3)              # [128, KC, B]
    ft_pieces = [
        np.ascontiguousarray(ft_kpn[:, :, q * NW : (q + 1) * NW]) for q in range(NFP)
    ]
    in_maps = []
    for i in range(NCORES):
        wpad = np.zeros((CSHP, D), dtype=np.float64)
        wsh = w[i * CSH : (i + 1) * CSH]
        wpad[:CSH] = wsh / wn[i * CSH : (i + 1) * CSH, None]
        wt_kpn = _to_kpn((F8S * wpad.T).astype(np.float32)).astype(float8_e4m3)
        m = {f"ft8_{q}": ft_pieces[q] for q in range(NFP)}
        for n in range(NCH):
            wd = NW if n < 4 else NW4
            m[f"wt8_{n}"] = np.ascontiguousarray(wt_kpn[:, :, n * NW : n * NW + wd])
        in_maps.append(m)
    return in_maps


def host_finish(outs, f, lab_word2vec, lab_pinds, lengths):
    """outs: list of 8 dicts with sexp. Returns float32 loss."""
    f = np.asarray(f, dtype=np.float64)
    w = np.asarray(lab_word2vec, dtype=np.float64)
    pinds = np.asarray(lab_pinds, dtype=np.int64)
    lens = np.asarray(lengths, dtype=np.int64)

    # S_shift[b] = sum_c exp(30 cos - 30).  parts col b = clean sum (exp on
    # ACT for ACT_BLOCKS, else q^16 from the DVE op, scaled by e^-30 here);
    # col 16+b = c4 sum (always DVE).  The 60 zero-pad classes sit in the c4
    # chunk and contribute q(0)^16 * e^-30 each (dot exactly 0).
    k_eas = math.exp(-S)
    pad_term = (CSHP - CSH) * (EAS_G ** EAS_P) * k_eas
    s_shift = np.zeros(B, dtype=np.float64)
    for i in range(NCORES):
        pa = outs[i]["parts"].astype(np.float64)         # [128, 3*NBLK]
        per_block = np.empty((128, NBLK), dtype=np.float64)
        for b in range(NBLK):
            clean = pa[:, 2 * b] + pa[:, 2 * b + 1]
            if b not in ACT_BLOCKS:
                clean *= k_eas
            per_block[:, b] = clean + pa[:, 2 * NBLK + b] * k_eas - pad_term
        s_shift += per_block.T.reshape(B)                # b = m*128 + p

    # exact positive-class cosines on host
    fn = np.linalg.norm(f, axis=1)                       # [B]
    wn = np.linalg.norm(w, axis=1)                       # [C]
    wsel = w[pinds]                                      # [B, LMAX, D]
    dots = np.einsum("bd,bld->bl", f, wsel)              # [B, LMAX]
    cos = dots / np.maximum(fn[:, None] * wn[pinds], 1e-8)

    cos_m, sin_m = math.cos(M_MARGIN), math.sin(M_MARGIN)
    th = math.cos(math.pi - M_MARGIN)
    mm = math.sin(math.pi - M_MARGIN) * M_MARGIN
    sine = np.sqrt(np.clip(1.0 - cos * cos, 0.0, 1.0))
    phi = cos * cos_m - sine * sin_m
    phi = np.where(cos > th, phi, cos - mm)

    mask = (np.arange(LMAX)[None, :] < lens[:, None])    # [B, LMAX] bool
    # dedup: a class replaced once in the denominator even if in 2 slots
    dup = np.zeros_like(mask)
    for j in range(1, LMAX):
        for j2 in range(j):
            dup[:, j] |= mask[:, j2] & (pinds[:, j2] == pinds[:, j])
    uniq = mask & ~dup
    corr = (uniq * (np.exp(S * phi - S) - np.exp(S * cos - S))).sum(axis=1)
    z = S + np.log(s_shift + corr)                       # logsumexp, [B]
    pos_sum = (mask * (S * phi)).sum(axis=1)
    L = lens.astype(np.float64)
    per_sample = (L * z - pos_sum) / (L * L)
    return np.float32(per_sample.mean())


def kernel(f, labels, lab_word2vec, lab_pinds, lengths):
    nc = _get_graph()
    in_maps = make_in_maps(f, lab_word2vec)
    res = run_bass_kernel_spmd(nc, in_maps, core_ids=list(range(NCORES)))
    return host_finish(res.results, f, lab_word2vec, lab_pinds, lengths)


# revision 53
# speedup vs baseline: 1.3935x; 1.0140x over previous
"""ArcFace combined-margin loss kernel for 8 TRN2 NeuronCores.

Strategy
--------
reference: cos = (f @ w.T) / (|f||w|); phi = arcface(cos);
outputs = s*(labels*phi + (1-labels)*cos); loss = mean over rows of
-(sum of log_softmax(outputs) at lab_pinds, masked) / L^2.

labels is the multi-hot of (lab_pinds, lengths), so outputs differs from
s*cos only at <=8 entries/row.  The only O(B*C) work is the dense
sexp[b] = sum_c exp(30*cos[b,c] - 30); everything else is O(B*LMAX) or
O((B+C)*D) and runs on host in float64.

Device (per core, classes C-sharded 2500/core zero-padded to 2560):
  inputs are pre-normalized, pre-transposed fp8(e4m3, x16) operands
  prepared on host.  Main loop over 16 row-blocks x 5 class-chunks:
  fp8 DoubleRow matmuls (K=256/instr) accumulate dots into PSUM, and
  the ScalarE reads each PSUM bank directly with one Exp activation
  (scale 30/256, bias -30) whose accum_out produces the per-row
  partial sum.  Output is just sexp [128, 80] per core.

Host (numpy, float64): row norms of f and w, normalization + transpose
+ fp8 quantization of the matmul operands, exact positive-class cos
via gather, arcface margin, denominator correction (dedup'd), ragged
CE, mean.  No collectives (cross-core reduction of [2048] scalars
happens on host during unsharding).
"""

import math
import sys

import numpy as np
from ml_dtypes import float8_e4m3

for _p in ("/opt/trn_rl_repo",):
    if _p not in sys.path:
        sys.path.append(_p)

import concourse.bass as bass
import concourse.bacc as bacc
import concourse.mybir as mybir
import concourse.tile as tile
from concourse.bass_utils import run_bass_kernel_spmd
from contextlib import ExitStack

# ---- custom DVE op: fused exp-approx + row-sum ---------------------------
# exp(t) for t = (30/256)*dot - 30 as e^-30 * q(w)^16 with w = dot*C0 and
# q(w) = w^2 + C1*w + C2 a monic quadratic (log-domain weighted fit on the
# actual dot distribution; end-to-end loss error is fp8-dominated).  One
# 8-stage DVE pass per PSUM tile produces the per-row partial sums via the
# lane accumulator, replacing an ACT exp + DVE reduce pair.
import concourse.dve_ops as dve_ops
import concourse.dve_spec as dve_spec
from concourse.dve_spec import AluOp as _DAlu, Bin as _DBin, Spec as _DSpec
from concourse.dve_spec import Src0 as _DSrc0, C0 as _DC0, C1 as _DC1, C2 as _DC2
from concourse.dve_spec import sq as _dsq
from concourse.dve_uop import DveOpSpec as _DveOpSpec
from operator import add as _op_add

EAS_C0 = 0.01176986
EAS_B = 1.22731124
EAS_G = 0.99729751
EAS_P = 8


def _ref_exp16_sum(in0, in1, c0, c1, c2):
    x = in0.astype(np.float32)
    wv = x * np.float32(c0)
    q = (wv + np.float32(c1)) * wv + np.float32(c2)
    out = q ** EAS_P
    acc = out.reshape(out.shape[0], -1).sum(axis=-1, keepdims=True)
    return out.astype(np.float32), acc.astype(np.float32)


def _register_exp16_sum():
    name = "EXP16_SUM_ANT"
    for op in dve_ops.OPS:
        if op.name == name:
            return op
    _w = _DBin(_DAlu.MULTIPLY, _DSrc0, _DC0)
    _q = _DBin(_DAlu.ADD, _DBin(_DAlu.MULTIPLY, _DBin(_DAlu.ADD, _w, _DC1), _w), _DC2)
    body = _dsq(_dsq(_dsq(_q)))
    spec = _DSpec(
        body=body, accum=_op_add, accum_init=dve_spec.Zero,
        reference=_ref_exp16_sum,
    )
    opcode = dve_ops._CUSTOM_DVE_ROW_BASE + len(dve_ops.OPS)
    assert opcode < 0x20
    op = dve_ops.DveOp(name, spec, subdim=False, uops_sha={})
    dve_ops._SUB_OPCODE_FOR_NAME[name] = opcode
    dve_ops.OPS.append(op)
    dve_ops.CUSTOM_DVE_SPECS[name] = spec
    for ver in ("v3",):
        uops = dve_spec.lower(spec, ver=ver)
        op.uops_sha[ver] = _DveOpSpec(
            name=name, opcode=opcode, uops=uops, rd1_en=False
        ).sha(ver)
    return op


EXP16_SUM = _register_exp16_sum()

B, C, D, LMAX = 2048, 20000, 512, 8
NCORES = 8
CSH = C // NCORES          # 2500 real classes per core
NW4 = 464                  # c4 chunk width (2048+464 = 2512, 12 pad classes)
CSHP = 2048 + NW4          # padded class shard
NBLK = B // 128            # 16 row blocks
NW = 512                   # matmul N-chunk width (one fp32 PSUM bank)
NCH = 5                    # 4 full chunks + the narrow c4 chunk
KC = D // 128              # 4 contraction chunks
NFP = 4                    # f pieces / supergroups (4 row-blocks each)
# clean-tile consumers: 10 blocks on ACT, 6 on the fused DVE op
ACT_BLOCKS = frozenset((0, 1, 2, 4, 5, 6, 8, 9, 12, 13))
S = 30.0
M_MARGIN = 0.5

F32 = mybir.dt.float32
BF16 = mybir.dt.bfloat16
FP8 = mybir.dt.float8e4
F8S = 16.0                 # fp8 pre-scale per operand (dots carry 256x)

_GRAPH = None


def build_graph():
    nc = bacc.Bacc()
    ft_ext = [
        nc.declare_dram_parameter(f"ft8_{q}", [128, KC, NW], FP8, isOutput=False)
        for q in range(NFP)
    ]
    wt_ext = [
        nc.declare_dram_parameter(
            f"wt8_{n}", [128, KC, NW if n < 4 else NW4], FP8, isOutput=False
        )
        for n in range(NCH)
    ]
    parts_ext = nc.declare_dram_parameter("parts", [128, 3 * NBLK], F32, isOutput=True)

    AF = mybir.ActivationFunctionType

    with ExitStack() as ctx:
        tc = ctx.enter_context(tile.TileContext(nc))
        const = ctx.enter_context(tc.tile_pool(name="const", bufs=1))
        resident = ctx.enter_context(tc.tile_pool(name="resident", bufs=1))
        esp = ctx.enter_context(tc.tile_pool(name="esp", bufs=3))
        dummy = ctx.enter_context(tc.tile_pool(name="dummy", bufs=3))
        pmm = ctx.enter_context(tc.tile_pool(name="pmm", bufs=4, space="PSUM"))

        nbias = const.tile([128, 1], F32)
        nc.vector.memset(nbias[:], -S)

        fT = [resident.tile([128, KC, NW], FP8, name=f"fT{q}") for q in range(NFP)]
        wT = [
            resident.tile([128, KC, NW if n < 4 else NW4], FP8, name=f"wT{n}")
            for n in range(NCH)
        ]
        # level-1 partial sums: one bf16 value per 128-element quarter-chunk
        parts_t = resident.tile([128, 3 * NBLK], F32)

        # spread input DMA descriptor writes over all engine queues, in the
        # order the matmul stream consumes the pieces; the first f/w pieces
        # are split into k-halves so the first matmul can start sooner
        loads = [
            (wT[0][:], wt_ext[0][:, :, :]),
            (wT[1][:], wt_ext[1][:, :, :]),
            (fT[0][:], ft_ext[0][:, :, :]),
            (wT[4][:], wt_ext[4][:, :, :]),
            (wT[2][:], wt_ext[2][:, :, :]),
            (wT[3][:], wt_ext[3][:, :, :]),
            (fT[1][:], ft_ext[1][:, :, :]),
            (fT[2][:], ft_ext[2][:, :, :]),
            (fT[3][:], ft_ext[3][:, :, :]),
        ]
        queues = [nc.sync, nc.scalar, nc.gpsimd]
        for i, (dst, src) in enumerate(loads):
            queues[i % len(queues)].dma_start(dst, src)

        # Chunk stream per supergroup g (= f piece, 4 row blocks): four
        # block-aligned "clean" tiles (chunks 0-3 of one block) then one
        # "c4" tile (chunk 4 of each of the 4 blocks).  Clean tiles are
        # consumed either by ACT (Exp + accum_out) or by the fused DVE
        # EXP16_SUM op; c4 tiles always by DVE (4 sub-slices, one accum
        # column per block).  parts col b = clean sum, col 16+b = c4 sum.
        # half-tiles: 2 PSUM banks = 2 chunks, 4-deep ping-pong.  Clean half
        # (b, h) = chunks 2h, 2h+1 of block b -> one accum col 2b+h; nasty
        # half (g, nh) = c4 chunks of blocks 4g+2nh, 4g+2nh+1 -> one accum
        # col 32+b per block.
        def clean_half(g, i, h):
            b = NFP * g + i
            ps = pmm.tile([128, 2, NW], F32, tag="mm", name=f"ps_c{b}_{h}")
            for n in (2 * h, 2 * h + 1):
                for k2 in range(KC // 2):
                    nc.tensor.matmul(
                        ps[:, n - 2 * h, :],
                        fT[g][:, 2 * k2 : 2 * k2 + 2, i * 128 : (i + 1) * 128],
                        wT[n][:, 2 * k2 : 2 * k2 + 2, :],
                        start=(k2 == 0),
                        stop=(k2 == KC // 2 - 1),
                        perf_mode=mybir.MatmulPerfMode.DoubleRow,
                    )
            col = 2 * b + h
            if b in ACT_BLOCKS:
                ed = esp.tile([128, 2 * NW], BF16, tag="ed", name=f"ed_{b}_{h}")
                nc.scalar.activation(
                    ed[:], ps[:, :, :], AF.Exp,
                    bias=nbias[:], scale=S / (F8S * F8S),
                    accum_out=parts_t[:, col : col + 1],
                )
            else:
                dm = dummy.tile([128, 2 * NW], BF16, tag="dm", name=f"dm_{b}_{h}")
                nc.vector._custom_dve(
                    EXP16_SUM, out=dm[:], in0=ps[:, :, :],
                    s0=EAS_C0, s1=EAS_B, imm2=EAS_G,
                    accum_out=parts_t[:, col : col + 1],
                )

        def nasty_half(g, nh):
            ps = pmm.tile([128, 2, NW], F32, tag="mm", name=f"ps_n{g}_{nh}")
            for ii in range(2):
                i = 2 * nh + ii
                for k2 in range(KC // 2):
                    nc.tensor.matmul(
                        ps[:, ii, 0:NW4],
                        fT[g][:, 2 * k2 : 2 * k2 + 2, i * 128 : (i + 1) * 128],
                        wT[4][:, 2 * k2 : 2 * k2 + 2, :],
                        start=(k2 == 0),
                        stop=(k2 == KC // 2 - 1),
                        perf_mode=mybir.MatmulPerfMode.DoubleRow,
                    )
            dmn = dummy.tile([128, 2, NW], BF16, tag="dmn", name=f"dmn_{g}_{nh}")
            for ii in range(2):
                b = NFP * g + 2 * nh + ii
                nc.vector._custom_dve(
                    EXP16_SUM, out=dmn[:, ii, 0:NW4], in0=ps[:, ii, 0:NW4],
                    s0=EAS_C0, s1=EAS_B, imm2=EAS_G,
                    accum_out=parts_t[:, 2 * NBLK + b : 2 * NBLK + b + 1],
                )

        # interleaved half-tile schedules (A=ACT clean, D=DVE clean, N=nasty)
        for g in range(NFP):
            if g == 0:
                # DMA-starved start: run all h0 halves first (they need only
                # wT0+wT1+fT0, the first pieces to land), then wT4/wT2/wT3 work
                seq = [
                    ("c", 0, 0), ("c", 3, 0), ("c", 1, 0), ("c", 2, 0),
                    ("n", 0), ("c", 3, 1), ("c", 0, 1), ("n", 1),
                    ("c", 1, 1), ("c", 2, 1),
                ]
            elif g == 1:
                a1, a2, a3, dd = 0, 1, 2, 3
                seq = [
                    ("c", a1, 0), ("c", a1, 1), ("c", dd, 0), ("c", a2, 0),
                    ("n", 0), ("c", a2, 1), ("n", 1), ("c", a3, 0),
                    ("c", a3, 1), ("c", dd, 1),
                ]
            elif g == 2:
                a1, a2, d1, d2 = 0, 1, 2, 3
                seq = [
                    ("c", d1, 0), ("c", d1, 1), ("c", a1, 0), ("c", d2, 0),
                    ("n", 0), ("c", a1, 1), ("c", a2, 0), ("n", 1),
                    ("c", a2, 1), ("c", d2, 1),
                ]
            else:
                # end the stream on ACT consumers (shorter drain tail)
                a1, a2, d1, d2 = 0, 1, 2, 3
                seq = [
                    ("c", d1, 0), ("c", d1, 1), ("c", a1, 0), ("c", d2, 0),
                    ("n", 0), ("c", a1, 1), ("c", d2, 1), ("n", 1),
                    ("c", a2, 0), ("c", a2, 1),
                ]
            for item in seq:
                if item[0] == "c":
                    clean_half(g, item[1], item[2])
                else:
                    nasty_half(g, item[1])
            if g == 1:
                nc.gpsimd.dma_start(parts_ext[:, 0:16], parts_t[:, 0:16])
                nc.gpsimd.dma_start(
                    parts_ext[:, 2 * NBLK : 2 * NBLK + 8],
                    parts_t[:, 2 * NBLK : 2 * NBLK + 8],
                )
        nc.sync.dma_start(parts_ext[:, 16 : 2 * NBLK], parts_t[:, 16 : 2 * NBLK])
        nc.sync.dma_start(
            parts_ext[:, 2 * NBLK + 8 :], parts_t[:, 2 * NBLK + 8 :]
        )

    nc.finalize()
    return nc


def _get_graph():
    global _GRAPH
    if _GRAPH is None:
        _GRAPH = build_graph()
    return _GRAPH


def _to_kpn(xT):
    """[D, N] (d-major) -> [128, KC, N] with partition p = d % 128, k = d // 128."""
    Dd, N = xT.shape
    return np.ascontiguousarray(xT.reshape(KC, 128, N).transpose(1, 0, 2))


def make_in_maps(f, lab_word2vec, lab_pinds=None):
    f = np.asarray(f, dtype=np.float64)
    w = np.asarray(lab_word2vec, dtype=np.float64)
    fn = np.linalg.norm(f, axis=1)
    wn = np.linalg.norm(w, axis=1)
    fhatT = (F8S * (f / fn[:, None]).T).astype(np.float32)   # [D, B]
    ft_kpn = _to_kpn(fhatT).astype(float8_e4m3)              # [128, KC, B]
    ft_pieces = [
        np.ascontiguousarray(ft_kpn[:, :, q * NW : (q + 1) * NW]) for q in range(NFP)
    ]
    in_maps = []
    for i in range(NCORES):
        wpad = np.zeros((CSHP, D), dtype=np.float64)
        wsh = w[i * CSH : (i + 1) * CSH]
        wpad[:CSH] = wsh / wn[i * CSH : (i + 1) * CSH, None]
        wt_kpn = _to_kpn((F8S * wpad.T).astype(np.float32)).astype(float8_e4m3)
        m = {f"ft8_{q}": ft_pieces[q] for q in range(NFP)}
        for n in range(NCH):
            wd = NW if n < 4 else NW4
            m[f"wt8_{n}"] = np.ascontiguousarray(wt_kpn[:, :, n * NW : n * NW + wd])
        in_maps.append(m)
    return in_maps


def host_finish(outs, f, lab_word2vec, lab_pinds, lengths):
    """outs: list of 8 dicts with sexp. Returns float32 loss."""
    f = np.asarray(f, dtype=np.float64)
    w = np.asarray(lab_word2vec, dtype=np.float64)
    pinds = np.asarray(lab_pinds, dtype=np.int64)
    lens = np.asarray(lengths, dtype=np.int64)

    # S_shift[b] = sum_c exp(30 cos - 30).  parts col b = clean sum (exp on
    # ACT for ACT_BLOCKS, else q^16 from the DVE op, scaled by e^-30 here);
    # col 16+b = c4 sum (always DVE).  The 60 zero-pad classes sit in the c4
    # chunk and contribute q(0)^16 * e^-30 each (dot exactly 0).
    k_eas = math.exp(-S)
    pad_term = (CSHP - CSH) * (EAS_G ** EAS_P) * k_eas
    s_shift = np.zeros(B, dtype=np.float64)
    for i in range(NCORES):
        pa = outs[i]["parts"].astype(np.float64)         # [128, 3*NBLK]
        per_block = np.empty((128, NBLK), dtype=np.float64)
        for b in range(NBLK):
            clean = pa[:, 2 * b] + pa[:, 2 * b + 1]
            if b not in ACT_BLOCKS:
                clean *= k_eas
            per_block[:, b] = clean + pa[:, 2 * NBLK + b] * k_eas - pad_term
        s_shift += per_block.T.reshape(B)                # b = m*128 + p

    # exact positive-class cosines on host
    fn = np.linalg.norm(f, axis=1)                       # [B]
    wn = np.linalg.norm(w, axis=1)                       # [C]
    wsel = w[pinds]                                      # [B, LMAX, D]
    dots = np.einsum("bd,bld->bl", f, wsel)              # [B, LMAX]
    cos = dots / np.maximum(fn[:, None] * wn[pinds], 1e-8)

    cos_m, sin_m = math.cos(M_MARGIN), math.sin(M_MARGIN)
    th = math.cos(math.pi - M_MARGIN)
    mm = math.sin(math.pi - M_MARGIN) * M_MARGIN
    sine = np.sqrt(np.clip(1.0 - cos * cos, 0.0, 1.0))
    phi = cos * cos_m - sine * sin_m
    phi = np.where(cos > th, phi, cos - mm)

    mask = (np.arange(LMAX)[None, :] < lens[:, None])    # [B, LMAX] bool
    # dedup: a class replaced once in the denominator even if in 2 slots
    dup = np.zeros_like(mask)
    for j in range(1, LMAX):
        for j2 in range(j):
            dup[:, j] |= mask[:, j2] & (pinds[:, j2] == pinds[:, j])
    uniq = mask & ~dup
    corr = (uniq * (np.exp(S * phi - S) - np.exp(S * cos - S))).sum(axis=1)
    z = S + np.log(s_shift + corr)                       # logsumexp, [B]
    pos_sum = (mask * (S * phi)).sum(axis=1)
    L = lens.astype(np.float64)
    per_sample = (L * z - pos_sum) / (L * L)
    return np.float32(per_sample.mean())


def kernel(f, labels, lab_word2vec, lab_pinds, lengths):
    nc = _get_graph()
    in_maps = make_in_maps(f, lab_word2vec)
    res = run_bass_kernel_spmd(nc, in_maps, core_ids=list(range(NCORES)))
    return host_finish(res.results, f, lab_word2vec, lab_pinds, lengths)
